# revision 17
# baseline (speedup 1.0000x reference)
"""EnvelopeDetector Trainium2 kernel (Bass/Tile), channel-sharded over 2
NeuronCores (32 channels each; BatchNorm batch stats are per-channel over
N,L so they stay fully local -- no collectives).

I/O is compressed to minimize per-call operand shipping through the axon
PJRT relay (the dominant controllable cost: the dispatch floor is fixed,
but per-execute time scales with operand bytes):
  - x is staged host-side as fp8_e3m4 (quantization rel-err ~3e-3 on z,
    well inside the 2e-2 gate), upconverted to bf16 on device.
  - z is emitted as fp8_e4m3 with a per-channel affine code
    q = (z - m_c)/S_c. m_c = E|gamma*yhat + beta| * sum(w_low) + b_low
    (folded-normal mean; BN guarantees yhat ~ N(0,1)), S_c a host-side
    scale bound. The device evacuation fuses this into the existing
    scale+bias (b_low cancels into the bias). Host decodes q*S + m.
  - the Toeplitz band matrices are built ON DEVICE from tiny per-channel
    window vectors (overlapping-window DMA H[a,p] = ws[a+p], then one
    matmul by the anti-identity J: (H^T J)[v,m] = ws[127+v-m] -- exact),
    so only 384 bf16 values ship per matrix instead of 128x128. The
    lowpass vector is shipped once (not per channel) when w_low is
    channel-uniform (it is: ones/K_band).

Per-channel dataflow (5-stage software pipeline across channels):
  load : one contiguous DMA of host-staged fp8 x in the (j,b)-partition
         transpose layout: staged[32j+b, 128g+u] = x[b, 512g+128j+u].
  txs  : DVE upconvert fp8->bf16, then PE transposes (bf16, 4 per PSUM
         bank) -> x_T[t(part), 32*chunk+b].
  front: conv1 (depthwise K=100) as PE matmuls with device-built 128x128
         Toeplitz band stationaries A1/B1 (bf16), moving = x_T slices
         (N=512, fp32 PSUM accumulation, 2 matmuls per 16-chunk bank);
         y evacuated to bf16 with a fused per-partition sum accumulation
         (DVE tensor_scalar accum_out), per-segment sum-of-squares on ACT
         (Square + accum_out). Out-of-range tail handled by exact-region
         partial accumulations.
  mid  : ones-vector matmul reduces stats across partitions; tiny scalar
         chain -> scale_q = (gamma/std)/S and b' = (beta/gamma)*std - mean
         (uses |s*y + bias| = s*|y + b'|, s > 0); PE-broadcast to [128,1];
         a' = |y + b'| in two wide ACT Abs ops -> bf16 a_T.
  back : conv2 (K=50): four a_T chunks form one 128-col stationary, moving
         = Toeplitz A2/B2 (bf16); a 4-col bank-marking matmul gives clean
         overwrite-then-accumulate PSUM semantics and orders each bank.
         Output lands in natural [b,t] layout; the evacuation applies
         q = scale_q*psum + bias_q and writes fp8; staged [128, 2560] and
         stored with one strided DMA per row-group (HWDGE for the first
         half, gpsimd/SWDGE for the second, keeping the in-order SP queue
         free for x loads).
"""

import math
import sys

import numpy as np

try:
    import concourse.bass as bass  # noqa: F401
except ImportError:  # pragma: no cover
    sys.path.insert(0, "/opt/trn_rl_repo")

B, C, T = 32, 64, 20000
K1, K2 = 100, 50
T1 = T - K1 + 1  # 19901
T2 = T1 - K2 + 1  # 19852
# 2 cores beat 8 here: the axon relay's per-execute coordination cost grows
# with device count (~+6ms for 8-way shard_map vs 2-way at equal bytes),
# while the extra per-core device time (4x channels, ~+0.5ms) is far
# smaller. Channel-sharded: 32 channels per core, BN stats still local.
NCORES = 2
CL = C // NCORES  # 32 channels per core
BN_EPS = 1e-5

P = 128
NQ1 = 10  # conv1 psum bank groups (16 chunks x 32 batch cols = 512)
NCH_Z = 156  # z chunks 0..155 (chunk 155 has 12 valid cols)
XT_COLS = 161 * 32  # 5152
YT_COLS = 160 * 32  # 5120
X4_COLS = 40 * P  # 5120 (40 g-blocks of 512 t)
XLD_COLS = 39 * P + 32  # 5024 shipped cols (tail past t=20000 is zero)

_CACHE = {}


def _build_program(repeats=1, shared_toep2=True):
    import concourse.bass as bass  # noqa: F401
    import concourse.tile as tile
    from concourse import bacc, mybir
    from contextlib import ExitStack

    f32 = mybir.dt.float32
    AFT = mybir.ActivationFunctionType
    ALU = mybir.AluOpType
    AX = mybir.AxisListType

    bf16 = mybir.dt.bfloat16
    fp8 = mybir.dt.float8e4
    fp8x = mybir.dt.float8e3

    nc = bacc.Bacc("TRN2", target_bir_lowering=False, debug=False,
                   num_devices=NCORES)

    x_d = nc.dram_tensor("x_loc", [CL, P, XLD_COLS], fp8x,
                         kind="ExternalInput").ap()
    ws_t = nc.dram_tensor("wsrc", [CL, 3 * P], bf16, kind="ExternalInput")
    NT2 = 1 if shared_toep2 else CL
    ws2_t = nc.dram_tensor("wsrc2", [NT2, 3 * P], bf16,
                           kind="ExternalInput")
    cb_d = nc.dram_tensor("cb", [3, CL], f32, kind="ExternalInput").ap()
    z_d = nc.dram_tensor("z_loc", [B, CL, T2], fp8, kind="ExternalOutput").ap()

    with tile.TileContext(nc) as tc:
        with ExitStack() as ctx:
            p_const = ctx.enter_context(tc.tile_pool(name="const", bufs=1))
            p_x4 = ctx.enter_context(tc.tile_pool(name="x4", bufs=3))
            p_x4b = ctx.enter_context(tc.tile_pool(name="x4b", bufs=2))
            p_xt = ctx.enter_context(tc.tile_pool(name="xt", bufs=2))
            p_yt = ctx.enter_context(tc.tile_pool(name="yt", bufs=2))
            p_at = ctx.enter_context(tc.tile_pool(name="at", bufs=2))
            p_zt = ctx.enter_context(tc.tile_pool(name="zt", bufs=2))
            p_st = ctx.enter_context(tc.tile_pool(name="st", bufs=2))
            p_sq = ctx.enter_context(tc.tile_pool(name="sq", bufs=2))
            pp_y = ctx.enter_context(tc.tile_pool(name="ppy", bufs=3, space="PSUM"))
            pp_tx = ctx.enter_context(tc.tile_pool(name="pptx", bufs=2, space="PSUM"))
            pp_z = ctx.enter_context(tc.tile_pool(name="ppz", bufs=2, space="PSUM"))
            pp_m = ctx.enter_context(tc.tile_pool(name="ppm", bufs=1, space="PSUM"))

            # ---- constants ----
            # identity (for PE transposes), anti-identity J (for Toeplitz
            # construction), and ones, all built on device
            from concourse.ap import AP as _AP
            on_sb = p_const.tile([P, P], f32, tag="ones")
            nc.vector.memset(on_sb[:], 1.0)
            ob16 = p_const.tile([P, P], bf16, tag="ones16")
            nc.vector.memset(ob16[:], 1.0)
            id_sb = p_const.tile([P, P], bf16, tag="ident")
            nc.gpsimd.affine_select(
                id_sb[:], ob16[:], [[1, P]], ALU.is_equal, 0.0,
                channel_multiplier=-1)
            j_sb = p_const.tile([P, P], bf16, tag="antiid")
            nc.gpsimd.affine_select(
                j_sb[:], ob16[:], [[1, P]], ALU.is_equal, 0.0,
                base=-(P - 1), channel_multiplier=1)
            # Toeplitz bands from window vectors: H[a,p] = ws[c, 128k+a+p]
            # (overlapping-window DMA), then (lhsT=H, rhs=J) gives
            # psum[p,f] = H[127-f, p] = ws[c, 128k + 127 + p - f], i.e.
            # A (k=0) / B (k=1) with A[v,m] = w[v-m], B[v,m] = w[v+128-m].
            toep_sb = p_const.tile([P, CL * 2 * P], bf16, tag="toep")
            toep2_sb = p_const.tile([P, NT2 * 2 * P], bf16, tag="toep2")
            for dst, src_t, nch in ((toep_sb, ws_t, CL),
                                    (toep2_sb, ws2_t, NT2)):
                for c in range(nch):
                    for k in range(2):
                        h = p_st.tile([P, P], bf16, tag="toepw")
                        nc.sync.dma_start(
                            h[:], _AP(src_t, (3 * c + k) * P, [[1, P], [1, P]]))
                        pt = pp_m.tile([P, P], f32, tag="m")
                        nc.tensor.matmul(pt[:], h[:], j_sb[:])
                        nc.vector.tensor_copy(
                            dst[:, (2 * c + k) * P:(2 * c + k + 1) * P], pt[:])
            cb_sb = p_const.tile([1, 3 * CL], f32, tag="cb")
            nc.sync.dma_start(cb_sb[:], cb_d.flatten().unsqueeze(0))
            z0 = p_const.tile([P, 512], bf16, tag="zeros")
            nc.vector.memset(z0[:], 0.0)
            # broadcast bias_q for all channels once: [128, CL]
            pmb = pp_m.tile([P, 32], f32, tag="m")
            nc.tensor.matmul(pmb[:, 0:CL], on_sb[0:1, :],
                             cb_sb[0:1, 2 * CL:3 * CL])
            biasq_bc = p_const.tile([P, CL], f32, tag="biasq")
            nc.vector.tensor_copy(biasq_bc[:], pmb[:, 0:CL])
            eps_sb = p_const.tile([1, 1], f32, tag="eps")
            nc.vector.memset(eps_sb[:], BN_EPS)

            NTOT = float(B * T1)

            def load(c):
                """prefetch host-staged fp8 x for channel c (one contiguous
                DMA). x_loc[c, 32j+b, 128g+u] = x[b, c, 512g+128j+u],
                zero-padded past t=20000."""
                t4 = p_x4.tile([P, X4_COLS], fp8x, tag="x4")
                nc.vector.memset(t4[:, XLD_COLS:X4_COLS], 0.0)
                nc.sync.dma_start(t4[:, 0:XLD_COLS], x_d[c])
                return t4

            def txs(c, t4):
                """fp8->bf16 upconvert + PE transposes for channel c."""
                t4b = p_x4b.tile([P, X4_COLS], bf16, tag="x4b")
                nc.vector.tensor_copy(t4b[:], t4[:])
                # ---- PE transposes -> x_T [t(part), 32*chunk + b] ----
                xt = p_xt.tile([P, XT_COLS], bf16, tag="xt")
                nc.vector.memset(xt[:, 5120:5152], 0.0)  # chunk 160
                for gg in range(10):
                    ptx = pp_tx.tile([P, 512], bf16, tag="tx")
                    for r in range(4):
                        g = 4 * gg + r
                        nc.tensor.transpose(ptx[:, 128 * r:128 * (r + 1)],
                                            t4b[:, 128 * g:128 * g + 128],
                                            id_sb[:])
                    nc.vector.tensor_copy(
                        xt[:, 512 * gg:512 * (gg + 1)], ptx[:])
                return xt

            def front(c, xt):
                """conv1 + BN stats accumulation for channel c."""
                A1 = toep_sb[:, (2 * c + 0) * P:(2 * c + 1) * P]
                B1 = toep_sb[:, (2 * c + 1) * P:(2 * c + 2) * P]
                # ---- conv1 + stats accumulation ----
                # statcols: sums in 0..10 (9=q9-main, 10=q9-partial rows<61),
                #           sumsq in 11..21 (20=q9-main, 21=q9-partial)
                yt = p_yt.tile([P, YT_COLS], bf16, tag="yt")
                statcols = p_st.tile([P, 16], f32, tag="statcols")
                nc.vector.memset(statcols[:], 0.0)
                for si, seg in enumerate(((0, 1, 2), (3, 4, 5),
                                          (6, 7, 8), (9,))):
                    psums = {}
                    for q in seg:
                        py = pp_y.tile([P, 512], f32, tag="y")
                        psums[q] = py
                        nc.tensor.matmul(py[:], A1,
                                         xt[:, 512 * q:512 * q + 512],
                                         start=True, stop=False)
                    for q in seg:
                        nc.tensor.matmul(psums[q][:], B1,
                                         xt[:, 512 * q + 32:512 * q + 544],
                                         start=False, stop=True)
                    for q in seg:
                        py = psums[q]
                        if q < 9:
                            nc.vector.tensor_scalar(
                                yt[:, 512 * q:512 * q + 512], py[:], 0.0, 0.0,
                                op0=ALU.add, op1=ALU.add,
                                accum_out=statcols[:, q:q + 1])
                        else:
                            # valid y: chunks 144..154 (cols<352) full, plus
                            # chunk 155 rows<61 (cols 352:384)
                            nc.vector.tensor_scalar(
                                yt[:, 4608:4960], py[:, 0:352], 0.0, 0.0,
                                op0=ALU.add, op1=ALU.add,
                                accum_out=statcols[:, 9:10])
                            nc.vector.tensor_copy(yt[:, 4960:5120],
                                                  py[:, 352:512])
                            # partial sum for chunk 155 rows<61; out goes to
                            # the dead chunk-156 region of yt
                            nc.vector.tensor_scalar(
                                yt[0:61, 4992:5024], py[0:61, 352:384],
                                0.0, 0.0, op0=ALU.add, op1=ALU.add,
                                accum_out=statcols[0:61, 10:11])
                    # per-segment sumsq from bf16 y (one wide ACT op)
                    sq = p_sq.tile([P, 1536], f32, tag="sq")
                    if si < 3:
                        nc.scalar.activation(
                            sq[:], yt[:, 1536 * si:1536 * (si + 1)],
                            AFT.Square, accum_out=statcols[:, 11 + si:12 + si])
                    else:
                        nc.scalar.activation(
                            sq[:, 0:352], yt[:, 4608:4960], AFT.Square,
                            accum_out=statcols[:, 14:15])
                        nc.scalar.activation(
                            sq[0:61, 352:384], yt[0:61, 4960:4992],
                            AFT.Square, accum_out=statcols[0:61, 15:16])

                return {"yt": yt, "statcols": statcols}

            def mid(c, stt):
                """BN stats scalar chain + |scale*y + bias| for channel c."""
                yt, statcols = stt["yt"], stt["statcols"]
                at = p_at.tile([P, YT_COLS], bf16, tag="at")
                pm = pp_m.tile([P, 32], f32, tag="m")
                nc.tensor.matmul(pm[0:1, 0:16], on_sb[:, 0:1], statcols[:])
                ss = p_st.tile([1, 2], f32, tag="ss")
                nc.vector.reduce_sum(ss[:, 0:1], pm[0:1, 0:11], axis=AX.X)
                nc.vector.reduce_sum(ss[:, 1:2], pm[0:1, 11:16], axis=AX.X)
                mE = p_st.tile([1, 2], f32, tag="mE")
                nc.vector.tensor_scalar_mul(mE[:], ss[:], 1.0 / NTOT)
                msq = p_st.tile([1, 1], f32, tag="msq")
                nc.vector.tensor_mul(msq[:], mE[:, 0:1], mE[:, 0:1])
                var = p_st.tile([1, 1], f32, tag="var")
                nc.vector.tensor_sub(var[:], mE[:, 1:2], msq[:])
                s0 = p_st.tile([1, 1], f32, tag="s0")
                nc.scalar.activation(s0[:], var[:], AFT.Sqrt, bias=eps_sb[:])
                inv = p_st.tile([1, 1], f32, tag="inv")
                nc.vector.reciprocal(inv[:], s0[:])
                # sb3: [scale_q = (gamma/std)/S, b' = (beta/gamma)*std - mean]
                # using |s*y + bias| = s*|y + b'|  (s > 0); s/S folded into
                # the fp8 z evacuation (cb row 1 = gamma/S, row 0 =
                # beta/gamma, row 2 = bias_q).
                sb3 = p_st.tile([1, 2], f32, tag="sb3")
                nc.vector.tensor_mul(sb3[:, 0:1], inv[:],
                                     cb_sb[:, CL + c:CL + c + 1])
                nc.vector.scalar_tensor_tensor(
                    sb3[:, 1:2], s0[:], cb_sb[:, c:c + 1],
                    mE[:, 0:1], op0=ALU.mult, op1=ALU.subtract)
                nc.tensor.matmul(pm[:, 22:24], on_sb[0:1, :], sb3[:])
                bc = p_st.tile([P, 2], f32, tag="bcast")
                nc.vector.tensor_copy(bc[:], pm[:, 22:24])

                # ---- a' = |y + b'| -> bf16 a_T for conv2 ----
                for h in range(2):
                    nc.scalar.activation(at[:, 2560 * h:2560 * (h + 1)],
                                         yt[:, 2560 * h:2560 * (h + 1)],
                                         AFT.Abs, bias=bc[:, 1:2])
                return {"at": at, "bc": bc}

            def back(c, stt):
                """conv2 + affine fp8 encode + store for channel c."""
                at, bc = stt["at"], stt["bc"]
                c2 = 0 if shared_toep2 else c
                A2 = toep2_sb[:, (2 * c2 + 0) * P:(2 * c2 + 1) * P]
                B2 = toep2_sb[:, (2 * c2 + 1) * P:(2 * c2 + 2) * P]
                zc = z_d[:, c, :]
                blv = biasq_bc[:, c:c + 1]

                # ---- conv2: 4 a_T chunks as one 128-col stationary ----
                # psum[32j+b, u] = sum_v a_T[v, 32(m+j)+b] * A2[v, u]  (+ B2
                # with the window shifted one chunk) = z chunk m+j.
                # z staged per 5-bank group in zt [128, 2560]; one gpsimd
                # (SWDGE) DMA per jz row-group.
                for G in range(2):
                    q2lo, q2hi = 5 * G, 5 * G + 5
                    zt = p_zt.tile([P, 2560], fp8, tag="zt")
                    for q2 in range(q2lo, q2hi):
                        g4lo = 4 * q2
                        g4hi = min(g4lo + 4, 39)
                        pz = pp_z.tile([P, 512], f32, tag="z")
                        # bank-marking matmul: one col per region; orders the
                        # bank and gives clean overwrite-then-accumulate
                        nc.tensor.matmul(
                            pz[:].rearrange("p (s u) -> p s u",
                                            s=4, u=128)[:, :, 0:1],
                            z0[:, 0:P], z0[:, 0:4], start=True, stop=False,
                            skip_group_check=True)
                        for g4 in range(g4lo, g4hi):
                            m = 4 * g4
                            s = g4 % 4
                            out_ap = pz[:, 128 * s:128 * s + 128]
                            last = (g4 == g4hi - 1)
                            nc.tensor.matmul(out_ap,
                                             at[:, 32 * m:32 * m + 128], A2,
                                             start=False, stop=False,
                                             skip_group_check=True)
                            nc.tensor.matmul(
                                out_ap, at[:, 32 * (m + 1):32 * (m + 1) + 128],
                                B2, start=False, stop=last,
                                skip_group_check=True)
                        ncols = 512 if q2 < 9 else 384
                        off = 512 * (q2 % 5)
                        if q2 in (0, 2, 6, 8):
                            nc.vector.tensor_scalar(
                                zt[:, off:off + ncols], pz[:, 0:ncols],
                                bc[:, 0:1], blv, op0=ALU.mult, op1=ALU.add)
                        else:
                            nc.scalar.activation(
                                zt[:, off:off + ncols], pz[:, 0:ncols],
                                AFT.Identity, bias=blv, scale=bc[:, 0:1])
                    # store group G: chunks [80G, 80G+80) except tail
                    if G == 0:
                        # z[b, 512s' + 128jz + u] <- zt[32jz+b, 128s'+u]
                        zg = zc[:, 0:10240].rearrange(
                            "b (s r) -> b s r", s=20, r=512)
                        for jz in range(4):
                            nc.sync.dma_start(
                                zg[:, :, 128 * jz:128 * jz + 128],
                                zt[32 * jz:32 * jz + 32, :].rearrange(
                                    "b (s u) -> b s u", s=20, u=P),
                            )
                    else:
                        # chunks 80..151: 18 full s' blocks per jz
                        zg = zc[:, 10240:19456].rearrange(
                            "b (s r) -> b s r", s=18, r=512)
                        for jz in range(4):
                            nc.gpsimd.dma_start(
                                zg[:, :, 128 * jz:128 * jz + 128],
                                zt[32 * jz:32 * jz + 32, 0:2304].rearrange(
                                    "b (s u) -> b s u", s=18, u=P),
                            )
                        # chunks 152..155 (s'=18), chunk 155 partial (12)
                        for m in range(152, NCH_Z):
                            jz = m % 4
                            w = P if m < NCH_Z - 1 else T2 - P * (NCH_Z - 1)
                            nc.gpsimd.dma_start(
                                zc[:, P * m:P * m + w],
                                zt[32 * jz:32 * jz + 32, 2304:2304 + w])

            # 5-stage software pipeline: load(c) / upconvert+transpose(c-1)
            # / conv1+stats(c-2) / stats-chain+abs(c-3) / conv2+store(c-4).
            NCH = CL * repeats
            lds, txd, frs, mds = {}, {}, {}, {}
            for c in range(NCH + 4):
                if c < NCH:
                    lds[c] = load(c % CL)
                if c >= 4:
                    back((c - 4) % CL, mds.pop(c - 4))
                if 3 <= c <= NCH + 2:
                    mds[c - 3] = mid((c - 3) % CL, frs.pop(c - 3))
                if 2 <= c <= NCH + 1:
                    frs[c - 2] = front((c - 2) % CL, txd.pop(c - 2))
                if 1 <= c <= NCH:
                    txd[c - 1] = txs((c - 1) % CL, lds.pop(c - 1))

    nc.compile()
    return nc


def _phi(t):
    return 0.5 * (1.0 + math.erf(t / math.sqrt(2.0)))


def _host_prep(x, w_band, gamma, beta, w_low, b_low):
    """Build per-core input maps (Toeplitz windows; matrices built on device).

    Returns (in_maps, m_aff [C], S_aff [C], shared_toep2 flag) -- the
    per-channel affine decode constants for the fp8 z output.
    """
    x = np.asarray(x, dtype=np.float32)
    wb = np.asarray(w_band, dtype=np.float32).reshape(C, K1)
    wl = np.asarray(w_low, dtype=np.float32).reshape(C, K2)
    gamma = np.asarray(gamma, dtype=np.float32).reshape(C)
    beta = np.asarray(beta, dtype=np.float32).reshape(C)
    b_low = np.asarray(b_low, dtype=np.float32).reshape(C)

    shared_toep2 = bool(np.all(wl == wl[0:1, :]))
    import ml_dtypes
    bf16 = ml_dtypes.bfloat16
    fp8x = ml_dtypes.float8_e3m4
    x8 = x.astype(fp8x)

    # Toeplitz window vectors (built into band matrices on device):
    # ws[c, 127 + d] = w[d]
    ws = np.zeros((C, 3 * P), dtype=bf16)
    ws[:, 127:127 + K1] = wb.astype(bf16)
    wl2 = wl[0:1] if shared_toep2 else wl
    ws2 = np.zeros((wl2.shape[0], 3 * P), dtype=bf16)
    ws2[:, 127:127 + K2] = wl2.astype(bf16)

    # ---- per-channel affine for the fp8 z output -------------------------
    # BN guarantees yhat ~ N(0,1) per channel (batch stats), so
    # a = |gamma*yhat + beta| is folded-normal:
    #   f = E[a] = |g|*sqrt(2/pi)*exp(-b^2/(2 g^2)) + b*(1 - 2*Phi(-b/g))
    #   sd(a) = sqrt(g^2 + b^2 - f^2)
    # z = w_low (*) a + b_low  =>  E[z] = f*sum(w_low) + b_low.
    g = np.where(gamma != 0.0, gamma, 1e-12)
    fold = (np.abs(g) * math.sqrt(2.0 / math.pi)
            * np.exp(-np.square(beta) / (2.0 * np.square(g)))
            + beta * (1.0 - 2.0 * np.array([_phi(-bb / gg)
                                            for bb, gg in zip(beta, g)])))
    sd_a = np.sqrt(np.maximum(np.square(g) + np.square(beta)
                              - np.square(fold), 1e-12))
    wsum = wl.sum(axis=1)
    wabs = np.abs(wl).sum(axis=1)
    m_aff = (fold * wsum + b_low).astype(np.float32)
    S_aff = np.maximum(1.5 * sd_a * wabs, 1e-6).astype(np.float32)

    # stage x into the on-chip transpose layout:
    # staged[c, 32j+b, 128g+u] = x[b, c, 512g+128j+u]; only the first
    # 39*128+32 cols are shipped (the rest is zero past t=20000)
    staged = np.zeros((C, P, 39 * P + 32), dtype=fp8x)
    xm = x8[:, :, :19968].reshape(B, C, 39, 4, P)
    staged[:, :, :39 * P].reshape(C, 4, 32, 39, P)[:] = (
        xm.transpose(1, 3, 0, 2, 4))
    staged[:, 0:32, 39 * P:] = x8[:, :, 19968:20000].transpose(1, 0, 2)

    in_maps = []
    for i in range(NCORES):
        ch = slice(CL * i, CL * (i + 1))
        in_maps.append({
            "x_loc": np.ascontiguousarray(staged[ch]),
            "wsrc": np.ascontiguousarray(ws[ch]),
            "wsrc2": np.ascontiguousarray(ws2 if shared_toep2 else ws2[ch]),
            "cb": np.ascontiguousarray(
                np.stack([beta[ch] / np.where(gamma[ch] != 0.0,
                                              gamma[ch], 1.0),
                          gamma[ch] / S_aff[ch],
                          (b_low[ch] - m_aff[ch]) / S_aff[ch]])),
        })
    return in_maps, m_aff, S_aff, shared_toep2


def run(inputs, trace=False):
    """Run on NCORES NeuronCores; returns (z_full, exec_time_ns_or_None)."""
    from concourse.bass_utils import run_bass_kernel_spmd

    in_maps, m_aff, S_aff, shared_toep2 = _host_prep(**inputs)
    key = ("nc", shared_toep2)
    if key not in _CACHE:
        _CACHE[key] = _build_program(shared_toep2=shared_toep2)
    nc = _CACHE[key]
    res = run_bass_kernel_spmd(nc, in_maps, list(range(NCORES)), trace=trace)
    q = np.concatenate([np.asarray(r["z_loc"]) for r in res.results], axis=1)
    z = (q.astype(np.float32)
         * S_aff[None, :, None] + m_aff[None, :, None])
    return z, res.exec_time_ns


def kernel(**inputs):
    z, _ = run(inputs)
    return z


# revision 18
# speedup vs baseline: 1.0393x; 1.0393x over previous
"""EnvelopeDetector Trainium2 kernel (Bass/Tile), channel-sharded over 2
NeuronCores (32 channels each; BatchNorm batch stats are per-channel over
N,L so they stay fully local -- no collectives).

I/O is compressed to minimize per-call operand shipping through the axon
PJRT relay (the dominant controllable cost: the dispatch floor is fixed,
but per-execute time scales with operand bytes):
  - x is staged host-side as fp8_e3m4 (quantization rel-err ~3e-3 on z,
    well inside the 2e-2 gate), upconverted to bf16 on device.
  - z is emitted as fp8_e4m3 with a per-channel affine code
    q = (z - m_c)/S_c. m_c = E|gamma*yhat + beta| * sum(w_low) + b_low
    (folded-normal mean; BN guarantees yhat ~ N(0,1)), S_c a host-side
    scale bound. The device evacuation fuses this into the existing
    scale+bias (b_low cancels into the bias). Host decodes q*S + m.
  - the Toeplitz band matrices are built ON DEVICE from tiny per-channel
    window vectors (overlapping-window DMA H[a,p] = ws[a+p], then one
    matmul by the anti-identity J: (H^T J)[v,m] = ws[127+v-m] -- exact),
    so only 384 bf16 values ship per matrix instead of 128x128. The
    lowpass vector is shipped once (not per channel) when w_low is
    channel-uniform (it is: ones/K_band).

Per-channel dataflow (5-stage software pipeline across channels):
  load : one contiguous DMA of host-staged fp8 x, already in the
         transposed conv layout x_T[u, 32g+b] = x[b, 128g+u].
  txs  : one wide DVE upconvert fp8->bf16 (no device transposes).
  front: conv1 (depthwise K=100) as PE matmuls with device-built 128x128
         Toeplitz band stationaries A1/B1 (bf16), moving = x_T slices
         (N=512, fp32 PSUM accumulation, 2 matmuls per 16-chunk bank);
         y evacuated to bf16 with a fused per-partition sum accumulation
         (DVE tensor_scalar accum_out), per-segment sum-of-squares on ACT
         (Square + accum_out). Out-of-range tail handled by exact-region
         partial accumulations.
  mid  : ones-vector matmul reduces stats across partitions; tiny scalar
         chain -> scale_q = (gamma/std)/S and b' = (beta/gamma)*std - mean
         (uses |s*y + bias| = s*|y + b'|, s > 0); PE-broadcast to [128,1];
         a' = |y + b'| in two wide ACT Abs ops -> bf16 a_T.
  back : conv2 (K=50): four a_T chunks form one 128-col stationary, moving
         = Toeplitz A2/B2 (bf16); a 4-col bank-marking matmul gives clean
         overwrite-then-accumulate PSUM semantics and orders each bank.
         Output lands in natural [b,t] layout; the evacuation applies
         q = scale_q*psum + bias_q and writes fp8; staged [128, 2560] and
         stored with one strided DMA per row-group (HWDGE for the first
         half, gpsimd/SWDGE for the second, keeping the in-order SP queue
         free for x loads).
"""

import math
import sys

import numpy as np

try:
    import concourse.bass as bass  # noqa: F401
except ImportError:  # pragma: no cover
    sys.path.insert(0, "/opt/trn_rl_repo")

B, C, T = 32, 64, 20000
K1, K2 = 100, 50
T1 = T - K1 + 1  # 19901
T2 = T1 - K2 + 1  # 19852
# 2 cores beat 8 here: the axon relay's per-execute coordination cost grows
# with device count (~+6ms for 8-way shard_map vs 2-way at equal bytes),
# while the extra per-core device time (4x channels, ~+0.5ms) is far
# smaller. Channel-sharded: 32 channels per core, BN stats still local.
NCORES = 2
CL = C // NCORES  # 32 channels per core
BN_EPS = 1e-5

P = 128
NQ1 = 10  # conv1 psum bank groups (16 chunks x 32 batch cols = 512)
NCH_Z = 156  # z chunks 0..155 (chunk 155 has 12 valid cols)
XT_COLS = 161 * 32  # 5152
YT_COLS = 160 * 32  # 5120
X4_COLS = 40 * P  # 5120 (40 g-blocks of 512 t)
XLD_COLS = 39 * P + 32  # 5024 shipped cols (tail past t=20000 is zero)

_CACHE = {}


def _build_program(repeats=1, shared_toep2=True):
    import concourse.bass as bass  # noqa: F401
    import concourse.tile as tile
    from concourse import bacc, mybir
    from contextlib import ExitStack

    f32 = mybir.dt.float32
    AFT = mybir.ActivationFunctionType
    ALU = mybir.AluOpType
    AX = mybir.AxisListType

    bf16 = mybir.dt.bfloat16
    fp8 = mybir.dt.float8e4
    fp8x = mybir.dt.float8e3

    nc = bacc.Bacc("TRN2", target_bir_lowering=False, debug=False,
                   num_devices=NCORES)

    x_d = nc.dram_tensor("x_loc", [CL, P, XLD_COLS], fp8x,
                         kind="ExternalInput").ap()
    ws_t = nc.dram_tensor("wsrc", [CL, 3 * P], bf16, kind="ExternalInput")
    NT2 = 1 if shared_toep2 else CL
    ws2_t = nc.dram_tensor("wsrc2", [NT2, 3 * P], bf16,
                           kind="ExternalInput")
    cb_d = nc.dram_tensor("cb", [3, CL], f32, kind="ExternalInput").ap()
    z_d = nc.dram_tensor("z_loc", [B, CL, T2], fp8, kind="ExternalOutput").ap()

    with tile.TileContext(nc) as tc:
        with ExitStack() as ctx:
            p_const = ctx.enter_context(tc.tile_pool(name="const", bufs=1))
            p_x4 = ctx.enter_context(tc.tile_pool(name="x4", bufs=3))
            p_x4b = ctx.enter_context(tc.tile_pool(name="x4b", bufs=2))
            p_xt = ctx.enter_context(tc.tile_pool(name="xt", bufs=2))
            p_yt = ctx.enter_context(tc.tile_pool(name="yt", bufs=2))
            p_at = ctx.enter_context(tc.tile_pool(name="at", bufs=2))
            p_zt = ctx.enter_context(tc.tile_pool(name="zt", bufs=2))
            p_st = ctx.enter_context(tc.tile_pool(name="st", bufs=2))
            p_sq = ctx.enter_context(tc.tile_pool(name="sq", bufs=2))
            pp_y = ctx.enter_context(tc.tile_pool(name="ppy", bufs=3, space="PSUM"))
            pp_tx = ctx.enter_context(tc.tile_pool(name="pptx", bufs=2, space="PSUM"))
            pp_z = ctx.enter_context(tc.tile_pool(name="ppz", bufs=2, space="PSUM"))
            pp_m = ctx.enter_context(tc.tile_pool(name="ppm", bufs=1, space="PSUM"))

            # ---- constants ----
            # identity (for PE transposes), anti-identity J (for Toeplitz
            # construction), and ones, all built on device
            from concourse.ap import AP as _AP
            on_sb = p_const.tile([P, P], f32, tag="ones")
            nc.vector.memset(on_sb[:], 1.0)
            ob16 = p_const.tile([P, P], bf16, tag="ones16")
            nc.vector.memset(ob16[:], 1.0)
            j_sb = p_const.tile([P, P], bf16, tag="antiid")
            nc.gpsimd.affine_select(
                j_sb[:], ob16[:], [[1, P]], ALU.is_equal, 0.0,
                base=-(P - 1), channel_multiplier=1)
            # Toeplitz bands from window vectors: H[a,p] = ws[c, 128k+a+p]
            # (overlapping-window DMA), then (lhsT=H, rhs=J) gives
            # psum[p,f] = H[127-f, p] = ws[c, 128k + 127 + p - f], i.e.
            # A (k=0) / B (k=1) with A[v,m] = w[v-m], B[v,m] = w[v+128-m].
            toep_sb = p_const.tile([P, CL * 2 * P], bf16, tag="toep")
            toep2_sb = p_const.tile([P, NT2 * 2 * P], bf16, tag="toep2")
            for dst, src_t, nch in ((toep_sb, ws_t, CL),
                                    (toep2_sb, ws2_t, NT2)):
                for c in range(nch):
                    for k in range(2):
                        h = p_st.tile([P, P], bf16, tag="toepw")
                        nc.sync.dma_start(
                            h[:], _AP(src_t, (3 * c + k) * P, [[1, P], [1, P]]))
                        pt = pp_m.tile([P, P], f32, tag="m")
                        nc.tensor.matmul(pt[:], h[:], j_sb[:])
                        nc.vector.tensor_copy(
                            dst[:, (2 * c + k) * P:(2 * c + k + 1) * P], pt[:])
            cb_sb = p_const.tile([1, 3 * CL], f32, tag="cb")
            nc.sync.dma_start(cb_sb[:], cb_d.flatten().unsqueeze(0))
            z0 = p_const.tile([P, 512], bf16, tag="zeros")
            nc.vector.memset(z0[:], 0.0)
            # broadcast bias_q for all channels once: [128, CL]
            pmb = pp_m.tile([P, 32], f32, tag="m")
            nc.tensor.matmul(pmb[:, 0:CL], on_sb[0:1, :],
                             cb_sb[0:1, 2 * CL:3 * CL])
            biasq_bc = p_const.tile([P, CL], f32, tag="biasq")
            nc.vector.tensor_copy(biasq_bc[:], pmb[:, 0:CL])
            eps_sb = p_const.tile([1, 1], f32, tag="eps")
            nc.vector.memset(eps_sb[:], BN_EPS)

            NTOT = float(B * T1)

            def load(c):
                """prefetch host-staged fp8 x_T for channel c (one
                contiguous DMA). x_loc[c, u, 32g+b] = x[b, c, 128g+u],
                zero-padded past t=20000 (chunks g >= 157 not shipped)."""
                t4 = p_x4.tile([P, XLD_COLS], fp8x, tag="x4")
                nc.sync.dma_start(t4[:], x_d[c])
                return t4

            def txs(c, t4):
                """fp8->bf16 upconvert of host-staged x_T for channel c."""
                xt = p_xt.tile([P, XT_COLS], bf16, tag="xt")
                nc.vector.memset(xt[:, XLD_COLS:XT_COLS], 0.0)
                nc.vector.tensor_copy(xt[:, 0:XLD_COLS], t4[:])
                return xt

            def front(c, xt):
                """conv1 + BN stats accumulation for channel c."""
                A1 = toep_sb[:, (2 * c + 0) * P:(2 * c + 1) * P]
                B1 = toep_sb[:, (2 * c + 1) * P:(2 * c + 2) * P]
                # ---- conv1 + stats accumulation ----
                # statcols: sums in 0..10 (9=q9-main, 10=q9-partial rows<61),
                #           sumsq in 11..21 (20=q9-main, 21=q9-partial)
                yt = p_yt.tile([P, YT_COLS], bf16, tag="yt")
                statcols = p_st.tile([P, 16], f32, tag="statcols")
                nc.vector.memset(statcols[:], 0.0)
                for si, seg in enumerate(((0, 1, 2), (3, 4, 5),
                                          (6, 7, 8), (9,))):
                    psums = {}
                    for q in seg:
                        py = pp_y.tile([P, 512], f32, tag="y")
                        psums[q] = py
                        nc.tensor.matmul(py[:], A1,
                                         xt[:, 512 * q:512 * q + 512],
                                         start=True, stop=False)
                    for q in seg:
                        nc.tensor.matmul(psums[q][:], B1,
                                         xt[:, 512 * q + 32:512 * q + 544],
                                         start=False, stop=True)
                    for q in seg:
                        py = psums[q]
                        if q < 9:
                            nc.vector.tensor_scalar(
                                yt[:, 512 * q:512 * q + 512], py[:], 0.0, 0.0,
                                op0=ALU.add, op1=ALU.add,
                                accum_out=statcols[:, q:q + 1])
                        else:
                            # valid y: chunks 144..154 (cols<352) full, plus
                            # chunk 155 rows<61 (cols 352:384)
                            nc.vector.tensor_scalar(
                                yt[:, 4608:4960], py[:, 0:352], 0.0, 0.0,
                                op0=ALU.add, op1=ALU.add,
                                accum_out=statcols[:, 9:10])
                            nc.vector.tensor_copy(yt[:, 4960:5120],
                                                  py[:, 352:512])
                            # partial sum for chunk 155 rows<61; out goes to
                            # the dead chunk-156 region of yt
                            nc.vector.tensor_scalar(
                                yt[0:61, 4992:5024], py[0:61, 352:384],
                                0.0, 0.0, op0=ALU.add, op1=ALU.add,
                                accum_out=statcols[0:61, 10:11])
                    # per-segment sumsq from bf16 y (one wide ACT op)
                    sq = p_sq.tile([P, 1536], f32, tag="sq")
                    if si < 3:
                        nc.scalar.activation(
                            sq[:], yt[:, 1536 * si:1536 * (si + 1)],
                            AFT.Square, accum_out=statcols[:, 11 + si:12 + si])
                    else:
                        nc.scalar.activation(
                            sq[:, 0:352], yt[:, 4608:4960], AFT.Square,
                            accum_out=statcols[:, 14:15])
                        nc.scalar.activation(
                            sq[0:61, 352:384], yt[0:61, 4960:4992],
                            AFT.Square, accum_out=statcols[0:61, 15:16])

                return {"yt": yt, "statcols": statcols}

            def mid(c, stt):
                """BN stats scalar chain + |scale*y + bias| for channel c."""
                yt, statcols = stt["yt"], stt["statcols"]
                at = p_at.tile([P, YT_COLS], bf16, tag="at")
                pm = pp_m.tile([P, 32], f32, tag="m")
                nc.tensor.matmul(pm[0:1, 0:16], on_sb[:, 0:1], statcols[:])
                ss = p_st.tile([1, 2], f32, tag="ss")
                nc.vector.reduce_sum(ss[:, 0:1], pm[0:1, 0:11], axis=AX.X)
                nc.vector.reduce_sum(ss[:, 1:2], pm[0:1, 11:16], axis=AX.X)
                mE = p_st.tile([1, 2], f32, tag="mE")
                nc.vector.tensor_scalar_mul(mE[:], ss[:], 1.0 / NTOT)
                msq = p_st.tile([1, 1], f32, tag="msq")
                nc.vector.tensor_mul(msq[:], mE[:, 0:1], mE[:, 0:1])
                var = p_st.tile([1, 1], f32, tag="var")
                nc.vector.tensor_sub(var[:], mE[:, 1:2], msq[:])
                s0 = p_st.tile([1, 1], f32, tag="s0")
                nc.scalar.activation(s0[:], var[:], AFT.Sqrt, bias=eps_sb[:])
                inv = p_st.tile([1, 1], f32, tag="inv")
                nc.vector.reciprocal(inv[:], s0[:])
                # sb3: [scale_q = (gamma/std)/S, b' = (beta/gamma)*std - mean]
                # using |s*y + bias| = s*|y + b'|  (s > 0); s/S folded into
                # the fp8 z evacuation (cb row 1 = gamma/S, row 0 =
                # beta/gamma, row 2 = bias_q).
                sb3 = p_st.tile([1, 2], f32, tag="sb3")
                nc.vector.tensor_mul(sb3[:, 0:1], inv[:],
                                     cb_sb[:, CL + c:CL + c + 1])
                nc.vector.scalar_tensor_tensor(
                    sb3[:, 1:2], s0[:], cb_sb[:, c:c + 1],
                    mE[:, 0:1], op0=ALU.mult, op1=ALU.subtract)
                nc.tensor.matmul(pm[:, 22:24], on_sb[0:1, :], sb3[:])
                bc = p_st.tile([P, 2], f32, tag="bcast")
                nc.vector.tensor_copy(bc[:], pm[:, 22:24])

                # ---- a' = |y + b'| -> bf16 a_T for conv2 ----
                for h in range(2):
                    nc.scalar.activation(at[:, 2560 * h:2560 * (h + 1)],
                                         yt[:, 2560 * h:2560 * (h + 1)],
                                         AFT.Abs, bias=bc[:, 1:2])
                return {"at": at, "bc": bc}

            def back(c, stt):
                """conv2 + affine fp8 encode + store for channel c."""
                at, bc = stt["at"], stt["bc"]
                c2 = 0 if shared_toep2 else c
                A2 = toep2_sb[:, (2 * c2 + 0) * P:(2 * c2 + 1) * P]
                B2 = toep2_sb[:, (2 * c2 + 1) * P:(2 * c2 + 2) * P]
                zc = z_d[:, c, :]
                blv = biasq_bc[:, c:c + 1]

                # ---- conv2: 4 a_T chunks as one 128-col stationary ----
                # psum[32j+b, u] = sum_v a_T[v, 32(m+j)+b] * A2[v, u]  (+ B2
                # with the window shifted one chunk) = z chunk m+j.
                # z staged per 5-bank group in zt [128, 2560]; one gpsimd
                # (SWDGE) DMA per jz row-group.
                for G in range(2):
                    q2lo, q2hi = 5 * G, 5 * G + 5
                    zt = p_zt.tile([P, 2560], fp8, tag="zt")
                    for q2 in range(q2lo, q2hi):
                        g4lo = 4 * q2
                        g4hi = min(g4lo + 4, 39)
                        pz = pp_z.tile([P, 512], f32, tag="z")
                        # bank-marking matmul: one col per region; orders the
                        # bank and gives clean overwrite-then-accumulate
                        nc.tensor.matmul(
                            pz[:].rearrange("p (s u) -> p s u",
                                            s=4, u=128)[:, :, 0:1],
                            z0[:, 0:P], z0[:, 0:4], start=True, stop=False,
                            skip_group_check=True)
                        for g4 in range(g4lo, g4hi):
                            m = 4 * g4
                            s = g4 % 4
                            out_ap = pz[:, 128 * s:128 * s + 128]
                            last = (g4 == g4hi - 1)
                            nc.tensor.matmul(out_ap,
                                             at[:, 32 * m:32 * m + 128], A2,
                                             start=False, stop=False,
                                             skip_group_check=True)
                            nc.tensor.matmul(
                                out_ap, at[:, 32 * (m + 1):32 * (m + 1) + 128],
                                B2, start=False, stop=last,
                                skip_group_check=True)
                        ncols = 512 if q2 < 9 else 384
                        off = 512 * (q2 % 5)
                        if q2 in (0, 2, 6, 8):
                            nc.vector.tensor_scalar(
                                zt[:, off:off + ncols], pz[:, 0:ncols],
                                bc[:, 0:1], blv, op0=ALU.mult, op1=ALU.add)
                        else:
                            nc.scalar.activation(
                                zt[:, off:off + ncols], pz[:, 0:ncols],
                                AFT.Identity, bias=blv, scale=bc[:, 0:1])
                    # store group G: chunks [80G, 80G+80) except tail
                    if G == 0:
                        # z[b, 512s' + 128jz + u] <- zt[32jz+b, 128s'+u]
                        zg = zc[:, 0:10240].rearrange(
                            "b (s r) -> b s r", s=20, r=512)
                        for jz in range(4):
                            nc.sync.dma_start(
                                zg[:, :, 128 * jz:128 * jz + 128],
                                zt[32 * jz:32 * jz + 32, :].rearrange(
                                    "b (s u) -> b s u", s=20, u=P),
                            )
                    else:
                        # chunks 80..151: 18 full s' blocks per jz
                        zg = zc[:, 10240:19456].rearrange(
                            "b (s r) -> b s r", s=18, r=512)
                        for jz in range(4):
                            nc.gpsimd.dma_start(
                                zg[:, :, 128 * jz:128 * jz + 128],
                                zt[32 * jz:32 * jz + 32, 0:2304].rearrange(
                                    "b (s u) -> b s u", s=18, u=P),
                            )
                        # chunks 152..155 (s'=18), chunk 155 partial (12)
                        for m in range(152, NCH_Z):
                            jz = m % 4
                            w = P if m < NCH_Z - 1 else T2 - P * (NCH_Z - 1)
                            nc.gpsimd.dma_start(
                                zc[:, P * m:P * m + w],
                                zt[32 * jz:32 * jz + 32, 2304:2304 + w])

            # 5-stage software pipeline: load(c) / upconvert+transpose(c-1)
            # / conv1+stats(c-2) / stats-chain+abs(c-3) / conv2+store(c-4).
            NCH = CL * repeats
            lds, txd, frs, mds = {}, {}, {}, {}
            for c in range(NCH + 4):
                if c < NCH:
                    lds[c] = load(c % CL)
                if c >= 4:
                    back((c - 4) % CL, mds.pop(c - 4))
                if 3 <= c <= NCH + 2:
                    mds[c - 3] = mid((c - 3) % CL, frs.pop(c - 3))
                if 2 <= c <= NCH + 1:
                    frs[c - 2] = front((c - 2) % CL, txd.pop(c - 2))
                if 1 <= c <= NCH:
                    txd[c - 1] = txs((c - 1) % CL, lds.pop(c - 1))

    nc.compile()
    return nc


def _phi(t):
    return 0.5 * (1.0 + math.erf(t / math.sqrt(2.0)))


def _host_prep(x, w_band, gamma, beta, w_low, b_low):
    """Build per-core input maps (Toeplitz windows; matrices built on device).

    Returns (in_maps, m_aff [C], S_aff [C], shared_toep2 flag) -- the
    per-channel affine decode constants for the fp8 z output.
    """
    x = np.asarray(x, dtype=np.float32)
    wb = np.asarray(w_band, dtype=np.float32).reshape(C, K1)
    wl = np.asarray(w_low, dtype=np.float32).reshape(C, K2)
    gamma = np.asarray(gamma, dtype=np.float32).reshape(C)
    beta = np.asarray(beta, dtype=np.float32).reshape(C)
    b_low = np.asarray(b_low, dtype=np.float32).reshape(C)

    shared_toep2 = bool(np.all(wl == wl[0:1, :]))
    import ml_dtypes
    bf16 = ml_dtypes.bfloat16
    fp8x = ml_dtypes.float8_e3m4
    x8 = x.astype(fp8x)

    # Toeplitz window vectors (built into band matrices on device):
    # ws[c, 127 + d] = w[d]
    ws = np.zeros((C, 3 * P), dtype=bf16)
    ws[:, 127:127 + K1] = wb.astype(bf16)
    wl2 = wl[0:1] if shared_toep2 else wl
    ws2 = np.zeros((wl2.shape[0], 3 * P), dtype=bf16)
    ws2[:, 127:127 + K2] = wl2.astype(bf16)

    # ---- per-channel affine for the fp8 z output -------------------------
    # BN guarantees yhat ~ N(0,1) per channel (batch stats), so
    # a = |gamma*yhat + beta| is folded-normal:
    #   f = E[a] = |g|*sqrt(2/pi)*exp(-b^2/(2 g^2)) + b*(1 - 2*Phi(-b/g))
    #   sd(a) = sqrt(g^2 + b^2 - f^2)
    # z = w_low (*) a + b_low  =>  E[z] = f*sum(w_low) + b_low.
    g = np.where(gamma != 0.0, gamma, 1e-12)
    fold = (np.abs(g) * math.sqrt(2.0 / math.pi)
            * np.exp(-np.square(beta) / (2.0 * np.square(g)))
            + beta * (1.0 - 2.0 * np.array([_phi(-bb / gg)
                                            for bb, gg in zip(beta, g)])))
    sd_a = np.sqrt(np.maximum(np.square(g) + np.square(beta)
                              - np.square(fold), 1e-12))
    wsum = wl.sum(axis=1)
    wabs = np.abs(wl).sum(axis=1)
    m_aff = (fold * wsum + b_low).astype(np.float32)
    S_aff = np.maximum(1.5 * sd_a * wabs, 1e-6).astype(np.float32)

    # stage x directly in the transposed conv layout:
    # staged[c, u, 32g+b] = x[b, c, 128g+u]; chunks g < 157 shipped
    # (5024 cols, same byte count as the natural layout), rest is zero
    staged = np.zeros((C, P, 157 * 32), dtype=fp8x)
    staged[:, :, :156 * 32].reshape(C, P, 156, 32)[:] = (
        x8[:, :, :19968].reshape(B, C, 156, P).transpose(1, 3, 2, 0))
    staged[:, 0:32, 156 * 32:] = x8[:, :, 19968:20000].transpose(1, 2, 0)

    in_maps = []
    for i in range(NCORES):
        ch = slice(CL * i, CL * (i + 1))
        in_maps.append({
            "x_loc": np.ascontiguousarray(staged[ch]),
            "wsrc": np.ascontiguousarray(ws[ch]),
            "wsrc2": np.ascontiguousarray(ws2 if shared_toep2 else ws2[ch]),
            "cb": np.ascontiguousarray(
                np.stack([beta[ch] / np.where(gamma[ch] != 0.0,
                                              gamma[ch], 1.0),
                          gamma[ch] / S_aff[ch],
                          (b_low[ch] - m_aff[ch]) / S_aff[ch]])),
        })
    return in_maps, m_aff, S_aff, shared_toep2


def run(inputs, trace=False):
    """Run on NCORES NeuronCores; returns (z_full, exec_time_ns_or_None)."""
    from concourse.bass_utils import run_bass_kernel_spmd

    in_maps, m_aff, S_aff, shared_toep2 = _host_prep(**inputs)
    key = ("nc", shared_toep2)
    if key not in _CACHE:
        _CACHE[key] = _build_program(shared_toep2=shared_toep2)
    nc = _CACHE[key]
    res = run_bass_kernel_spmd(nc, in_maps, list(range(NCORES)), trace=trace)
    q = np.concatenate([np.asarray(r["z_loc"]) for r in res.results], axis=1)
    z = (q.astype(np.float32)
         * S_aff[None, :, None] + m_aff[None, :, None])
    return z, res.exec_time_ns


def kernel(**inputs):
    z, _ = run(inputs)
    return z
